# revision 24
# baseline (speedup 1.0000x reference)
"""Trainium2 Bass kernel for nn_Block_22497038696617 (dense transformer block).

Block: pre-LN attention with policy-masked softmax + pre-LN MLP (exact GELU).
  B=2, N=2048, C=768, H=12 heads x 64, HID=3072, fp32 in/out.

Sharding (8 cores, zero cross-core communication, single SPMD launch):
  core c -> batch b = c//4, query block qoff = (c%4)*512.
  Each core computes LN1 + K/V for the full sequence of its batch
  (replicated across the 4 cores of that batch), Q/attention/proj/MLP for
  its own 512 query rows, and writes its [512, 768] output slice.
  Host gathers the 8 slices into the full [2, 2048, 768] output.

Key compaction + program-uniformity trick: attention is permutation-invariant
over keys, and a key whose policy bit is 0 contributes ~0 to every query (its
post-mask P is ~e-50) EXCEPT to its own query via the "always attend to self"
diagonal. So each core's key axis is rebuilt on the host as
  [its own 512 queries (always kept), all unmasked other keys, zero padding]
padded to a multiple of 512 (typically 1536 of the original 2048). This (a)
drops ~25% of the K/V/attention work, and (b) pins the diagonal exception to
k-tiles 0..3 at column offset t*128, making the SPMD program identical on all
cores even though each core has a different query block.

Matmuls run in float32r (TF32-like, ~1.5e-4 rel err, full PE speed at
free-dim >= 256) with fp32 PSUM accumulation. Softmax skips max-subtraction
(logits are O(1); fp32 exp is safe) and folds the policy mask into the exp as
a per-key bias of ln(policy) (0 or -50). The softmax denominator comes free
from a ones-column appended to V. The numerator +POL_EPS/n term (~5e-10
absolute on O(1e-3) values) is dropped as negligible; denominator +POL_EPS
is kept. Attention layout is transposed (S^T [keys, queries]) so softmax
masking is per-partition and no P transposes are needed. Heads are processed
in pairs sharing one [128, 1024] PSUM tile so each k-tile needs a single exp
instruction, and the two S-matmuls land in disjoint PE row groups (rows 0-63 /
64-127) and can execute concurrently.
"""

from contextlib import ExitStack

import numpy as np

import concourse.bacc as bacc
import concourse.mybir as mybir
import concourse.tile as tile
from concourse.bass_utils import run_bass_kernel_spmd

f32 = mybir.dt.float32
f32r = mybir.dt.float32r
AF = mybir.ActivationFunctionType
OP = mybir.AluOpType

B, N, C = 2, 2048, 768
H, HD = 12, 64
HID = 3072
NCORES = 8
QB = 512                 # own query rows per core
CT = C // 128            # 6 c-tiles
FT = C // 128            # 6 f-tiles (H*HD == C)
KTN = N // 128           # 16 k-tiles
HB = HID // 128          # 24 hid-tiles
SCALE = HD ** -0.5
LN_EPS = 1e-5
POL_EPS = 1e-6
MASK_NEG = -50.0

TRACE = False            # set True by the dev harness for profiling runs
TRACE_KWARGS = {}
LAST_RESULTS = None      # BassKernelResults of the last run (for timing)

_prog_cache = {}


def _build_program(ln1_triv, ln2_triv, projb_triv, fc2b_triv, kpad):
    kt_n = kpad // 128          # k-tiles after key compaction
    kq_n = kpad // 512          # 512-wide key chunks
    nc = bacc.Bacc("TRN2", target_bir_lowering=False, debug=False,
                   num_devices=NCORES)

    # ---- DRAM I/O (f32r dtype for tensors DMA'd straight into matmuls) ----
    xT_d = nc.dram_tensor("xT", [C, kpad], f32r, kind="ExternalInput")
    xown_d = nc.dram_tensor("x_own", [QB, C], f32, kind="ExternalInput")
    pol_d = nc.dram_tensor("pol", [128, kt_n], f32, kind="ExternalInput")
    lnp_d = nc.dram_tensor("lnp", [128, kt_n], f32, kind="ExternalInput")
    # weight packs: [p, ci, f] layout so one DMA loads a [128, ...] tile whose
    # [:, ci, :] slice is the lhsT/rhs tile for c-tile ci (contiguous lines)
    wq_d = nc.dram_tensor("wq_packT", [FT, 128, CT * 128], f32r,
                          kind="ExternalInput")
    wk_d = nc.dram_tensor("wk_packT", [FT, 128, CT * 128], f32r,
                          kind="ExternalInput")
    wv_d = nc.dram_tensor("wv_packT", [128, CT * C], f32r,
                          kind="ExternalInput")
    projw_d = nc.dram_tensor("projwT", [C, C], f32r, kind="ExternalInput")
    fc1w_d = nc.dram_tensor("fc1w_pack", [CT, 6, 128, 512], f32r,
                            kind="ExternalInput")
    fc2w_d = nc.dram_tensor("fc2wT", [HID, C], f32r, kind="ExternalInput")
    fc1b_d = nc.dram_tensor("fc1b", [128, HB], f32, kind="ExternalInput")
    eye_d = nc.dram_tensor("eye", [128, 128], f32, kind="ExternalInput")
    if not ln1_triv:
        ln1gb_d = nc.dram_tensor("ln1gb", [128, 2 * CT], f32,
                                 kind="ExternalInput")
    if not ln2_triv:
        ln2gb_d = nc.dram_tensor("ln2gb", [2, C], f32, kind="ExternalInput")
    if not projb_triv:
        projb_d = nc.dram_tensor("projb", [1, C], f32, kind="ExternalInput")
    if not fc2b_triv:
        fc2b_d = nc.dram_tensor("fc2b", [1, C], f32, kind="ExternalInput")
    yout_d = nc.dram_tensor("yout", [QB, C], f32, kind="ExternalOutput")

    with tile.TileContext(nc) as tc, ExitStack() as ctx:
        # ---------------- constants + whole-kernel persistents --------------
        pG = ctx.enter_context(tc.tile_pool(name="pG", bufs=1))
        eye_sb = pG.tile([128, 128], f32, name="eye_sb")
        nc.gpsimd.dma_start(out=eye_sb, in_=eye_d.ap())
        pol_sb = pG.tile([128, kt_n], f32, name="pol_sb")
        nc.gpsimd.dma_start(out=pol_sb, in_=pol_d.ap())
        lnp_sb = pG.tile([128, kt_n], f32, name="lnp_sb")
        nc.gpsimd.dma_start(out=lnp_sb, in_=lnp_d.ap())
        fc1b_sb = pG.tile([128, HB], f32, name="fc1b_sb")
        nc.gpsimd.dma_start(out=fc1b_sb, in_=fc1b_d.ap())
        invpol_sb = pG.tile([128, kt_n], f32, name="invpol_sb")
        nc.vector.tensor_scalar(invpol_sb, pol_sb, -1.0, 1.0,
                                op0=OP.mult, op1=OP.add)
        ones_col = pG.tile([128, 1], f32r, name="ones_col")
        nc.vector.memset(ones_col.bitcast(f32), 1.0)
        # attention output, transposed, per head-pair: OTp[j] rows = features
        # of heads (2j, 2j+1), cols = own queries
        OTp = [pG.tile([128, QB], f32r, name=f"otp{j}") for j in range(FT)]
        # attention-residual rows live here so proj (emitted inside the
        # attention scope) can write them and phase C can read them
        x_res = [pG.tile([128, C], f32, name=f"xres{st}") for st in range(4)]

        # ======================= phase A + B scope ==========================
        with tc.tile_pool(name="pAB", bufs=1) as pAB:
            KTp = [pAB.tile([128, kpad], f32r, name=f"ktp{j}") for j in range(FT)]
            QTp = [pAB.tile([128, QB], f32r, name=f"qtp{j}") for j in range(FT)]
            vpad = [pAB.tile([128, H, HD + 1], f32r, name=f"vpad{t}")
                    for t in range(kt_n)]

            # --------------- phase A: LN1 + QKV projections -----------------
            # Software-pipelined: quarter q's LN stats/apply overlap quarter
            # q-1's K/V/Q matmuls.
            with tc.tile_pool(name="pA", bufs=1) as pA, \
                 tc.tile_pool(name="psA", bufs=1, space="PSUM") as psA:
                if not ln1_triv:
                    ln1gb_sb = pA.tile([128, 2 * CT], f32, name="ln1gb_sb")
                    nc.sync.dma_start(out=ln1gb_sb, in_=ln1gb_d.ap())
                # resident V weights: [p, ci, f] single contiguous DMA
                wv_sb = pA.tile([128, CT, C], f32r, name="wv_sb")
                nc.gpsimd.dma_start(
                    out=wv_sb.rearrange("p a b -> p (a b)"), in_=wv_d.ap())

                def ln_loads_stats(qr):
                    """x.T loads + stats matmuls for one key chunk."""
                    s0 = qr * 512
                    xt = []
                    for ci in range(CT):
                        t_ = pA.tile([128, 512], f32r, name="xt", tag="xt",
                                     bufs=6)
                        nc.sync.dma_start(
                            out=t_,
                            in_=xT_d.ap()[ci * 128:(ci + 1) * 128, s0:s0 + 512])
                        xt.append(t_)
                    # stats via ones-matmuls (sum over c = partition dim)
                    ps_mean = psA.tile([1, 512], f32, name="ps_mean",
                                       tag="psmean", bufs=2)
                    ps_sq = psA.tile([1, 512], f32, name="ps_sq",
                                     tag="pssq", bufs=2)
                    for ci in range(CT):
                        nc.tensor.matmul(ps_mean, ones_col, xt[ci],
                                         start=(ci == 0), stop=(ci == CT - 1))
                    for ci in range(CT):
                        xsq = pA.tile([128, 512], f32r, name="xsq", tag="d_t",
                                      bufs=2)
                        nc.vector.tensor_mul(xsq, xt[ci].bitcast(f32),
                                             xt[ci].bitcast(f32))
                        nc.tensor.matmul(ps_sq, ones_col, xsq,
                                         start=(ci == 0), stop=(ci == CT - 1))
                    return xt, ps_mean, ps_sq

                def ln_rows_hl(qr, stage):
                    """LN1 row stats -> broadcast -> h_ln.T build."""
                    xt, ps_mean, ps_sq = stage
                    # rows: mean, var+eps, rstd = exp(-0.5*ln(var+eps))
                    def row(nm):
                        return pA.tile([1, 512], f32, name=nm, tag="rows",
                                       bufs=3)
                    mrow = row("mrow")
                    nc.vector.tensor_scalar_mul(mrow, ps_mean, 1.0 / C)
                    ve = row("ve")
                    nc.vector.tensor_scalar(ve, ps_sq, 1.0 / C, LN_EPS,
                                            op0=OP.mult, op1=OP.add)
                    m2 = row("m2")
                    nc.vector.tensor_mul(m2, mrow, mrow)
                    nc.vector.tensor_sub(ve, ve, m2)
                    nc.scalar.activation(ve, ve, AF.Ln)
                    r0 = row("r0")
                    nc.scalar.activation(r0, ve, AF.Exp, scale=-0.5)
                    bc_m = pA.tile([128, 512], f32, name="bc_m", tag="bc_m",
                                   bufs=1)
                    nc.gpsimd.partition_broadcast(bc_m, mrow)
                    bc_r = pA.tile([128, 512], f32, name="bc_r", tag="bc_r",
                                   bufs=1)
                    nc.gpsimd.partition_broadcast(bc_r, r0)
                    hl = []
                    for ci in range(CT):
                        d_t = pA.tile([128, 512], f32, name="d_t", tag="d_t",
                                      bufs=2)
                        nc.vector.tensor_sub(d_t, xt[ci].bitcast(f32), bc_m)
                        h_ = pA.tile([128, 512], f32r, name="hl", tag="hl",
                                     bufs=12)
                        if ln1_triv:
                            nc.vector.tensor_tensor(out=h_, in0=d_t, in1=bc_r,
                                                    op=OP.mult)
                        else:
                            nc.vector.tensor_tensor(out=d_t, in0=d_t, in1=bc_r,
                                                    op=OP.mult)
                            nc.vector.tensor_scalar(
                                h_, d_t, ln1gb_sb[:, ci:ci + 1],
                                ln1gb_sb[:, CT + ci:CT + ci + 1],
                                op0=OP.mult, op1=OP.add)
                        hl.append(h_)
                    return hl

                def kvq_stage(qr, hl):
                    """K/V (+Q for quarter 0) matmuls for one quarter."""
                    s0 = qr * 512
                    for fj in range(FT):
                        wk = pA.tile([128, CT, 128], f32r, name="wk", tag="wk",
                                     bufs=2)
                        nc.gpsimd.dma_start(
                            out=wk.rearrange("p a b -> p (a b)"),
                            in_=wk_d.ap()[fj])
                        psk = psA.tile([128, 512], f32, name="psk",
                                       tag="pskv", bufs=3)
                        for ci in range(CT):
                            nc.tensor.matmul(psk, wk[:, ci, :], hl[ci],
                                             start=(ci == 0),
                                             stop=(ci == CT - 1))
                        nc.vector.tensor_copy(KTp[fj][:, s0:s0 + 512], psk)
                    for si in range(4):
                        st = qr * 4 + si
                        for fc in range(2):
                            f0 = fc * 512
                            wsz = 512 if fc == 0 else 256
                            psv = psA.tile([128, 512], f32, name="psv",
                                           tag="pskv", bufs=3)
                            for ci in range(CT):
                                nc.tensor.matmul(
                                    psv[:, 0:wsz],
                                    hl[ci][:, si * 128:(si + 1) * 128],
                                    wv_sb[:, ci, f0:f0 + wsz],
                                    start=(ci == 0), stop=(ci == CT - 1))
                            nh = wsz // HD
                            h0 = 0 if fc == 0 else 8
                            nc.vector.tensor_copy(
                                vpad[st][:, h0:h0 + nh, 0:HD],
                                psv[:, 0:wsz].rearrange(
                                    "p (h d) -> p h d", h=nh))
                        nc.vector.memset(vpad[st].bitcast(f32)[:, :, HD], 1.0)
                    if qr == 0:
                        # own queries are keys 0:512 => Q.T from quarter 0
                        for fj in range(FT):
                            wq = pA.tile([128, CT, 128], f32r, name="wq",
                                         tag="wk", bufs=2)
                            nc.gpsimd.dma_start(
                                out=wq.rearrange("p a b -> p (a b)"),
                                in_=wq_d.ap()[fj])
                            psq = psA.tile([128, 512], f32, name="psq",
                                           tag="pskv", bufs=3)
                            for ci in range(CT):
                                nc.tensor.matmul(psq, wq[:, ci, :], hl[ci],
                                                 start=(ci == 0),
                                                 stop=(ci == CT - 1))
                            nc.vector.tensor_copy(QTp[fj], psq)

                # 2-deep software pipeline: PE order is
                #   stats(0), stats(1), kvq(0), stats(2), kvq(1), ...
                # so chunk q+1's stats matmuls (gated on DMA) issue while
                # chunk q's K/V inputs are already on-chip, and the LN row /
                # h_ln.T DVE work of q+1 overlaps kvq(q) on the PE.
                stage = ln_loads_stats(0)
                hl_prev = ln_rows_hl(0, stage)
                for qr in range(1, kq_n):
                    stage = ln_loads_stats(qr)
                    kvq_stage(qr - 1, hl_prev)
                    hl_prev = ln_rows_hl(qr, stage)
                kvq_stage(kq_n - 1, hl_prev)

            # --------------- phase B: attention (head pairs) ----------------
            with tc.tile_pool(name="pB", bufs=1) as pB, \
                 tc.tile_pool(name="psB", bufs=1, space="PSUM") as psB:
                projw = [pB.tile([128, C], f32r, name=f"pjw{fj}")
                         for fj in range(FT)]
                for fj in range(FT):
                    nc.sync.dma_start(
                        out=projw[fj],
                        in_=projw_d.ap()[fj * 128:(fj + 1) * 128, :])
                x_own = [pB.tile([128, C], f32, name=f"xown{st}")
                         for st in range(4)]
                for st in range(4):
                    nc.sync.dma_start(
                        out=x_own[st],
                        in_=xown_d.ap()[st * 128:(st + 1) * 128, :])
                for jp in range(H // 2):
                    h0, h1 = 2 * jp, 2 * jp + 1
                    ps_o0 = psB.tile([HD + 1, QB], f32, name="ps_o0",
                                     tag="pso", bufs=3)
                    ps_o1 = psB.tile([HD + 1, QB], f32, name="ps_o1",
                                     tag="pso", bufs=3)

                    def o_mms(ti, t, p_t):
                        nc.tensor.matmul(ps_o0, vpad[t][:, h0, :],
                                         p_t[:, 0:QB],
                                         start=(ti == 0), stop=(ti == kt_n - 1),
                                         skip_group_check=True)
                        nc.tensor.matmul(ps_o1, vpad[t][:, h1, :],
                                         p_t[:, QB:2 * QB],
                                         start=(ti == 0), stop=(ti == kt_n - 1),
                                         skip_group_check=True)

                    prev = None
                    # diag tiles (0..3) carry extra DVE mask work; process
                    # them LAST so a pair's first O-matmuls aren't gated on
                    # the DVE chain right at the pair boundary
                    t_order = list(range(4, kt_n)) + [0, 1, 2, 3]
                    for ti, t in enumerate(t_order):
                        ps_s = psB.tile([128, 2 * QB], f32, name="ps_s",
                                        tag="pss", bufs=2)
                        nc.tensor.matmul(
                            ps_s[:, 0:QB],
                            KTp[jp][0:64, t * 128:(t + 1) * 128],
                            QTp[jp][0:64, :],
                            start=True, stop=True, skip_group_check=True)
                        nc.tensor.matmul(
                            ps_s[:, QB:2 * QB],
                            KTp[jp][64:128, t * 128:(t + 1) * 128],
                            QTp[jp][64:128, :],
                            start=True, stop=True, skip_group_check=True)
                        p_t = pB.tile([128, 2 * QB], f32r, name="p_t",
                                      tag="pt", bufs=6)
                        if t >= 4:
                            # mask folded into exp: exp(scale*s + ln(policy))
                            nc.scalar.activation(p_t, ps_s, AF.Exp,
                                                 bias=lnp_sb[:, t:t + 1],
                                                 scale=SCALE)
                        else:
                            # diagonal k-tile: need exp * max(eye, policy).
                            # p = exp(s); dc = p*eye*(1-pol) on the two diag
                            # blocks; p *= pol; p += dc.
                            off = t * 128
                            nc.scalar.activation(p_t, ps_s, AF.Exp,
                                                 scale=SCALE)
                            cor = pB.tile([128, 128], f32, name="cor",
                                          tag="cor", bufs=2)
                            nc.vector.tensor_scalar_mul(
                                cor, eye_sb, invpol_sb[:, t:t + 1])
                            dc0 = pB.tile([128, 128], f32, name="dc0",
                                          tag="dc0", bufs=2)
                            nc.vector.tensor_tensor(
                                out=dc0, in0=p_t[:, off:off + 128].bitcast(f32),
                                in1=cor, op=OP.mult)
                            dc1 = pB.tile([128, 128], f32, name="dc1",
                                          tag="dc1", bufs=2)
                            nc.vector.tensor_tensor(
                                out=dc1,
                                in0=p_t[:, QB + off:QB + off + 128].bitcast(f32),
                                in1=cor, op=OP.mult)
                            nc.vector.tensor_scalar_mul(
                                p_t, p_t.bitcast(f32), pol_sb[:, t:t + 1])
                            nc.vector.tensor_tensor(
                                out=p_t[:, off:off + 128],
                                in0=p_t[:, off:off + 128].bitcast(f32),
                                in1=dc0, op=OP.add)
                            nc.vector.tensor_tensor(
                                out=p_t[:, QB + off:QB + off + 128],
                                in0=p_t[:, QB + off:QB + off + 128].bitcast(f32),
                                in1=dc1, op=OP.add)
                        # software pipeline: O matmuls trail by one k-tile so
                        # the PE can run S(t+1) while the ACT exp(t) finishes
                        if prev is not None:
                            o_mms(*prev)
                        prev = (ti, t, p_t)
                    o_mms(*prev)
                    # normalize: O / (denominator + POL_EPS); the reciprocal
                    # runs on ACT as exp(-ln(d)) so the in-order DVE queue
                    # isn't blocked ahead of the next pair's mask ops
                    for hh, ps_o in ((0, ps_o0), (64, ps_o1)):
                        # copy PSUM out immediately so the accumulator bank is
                        # released for the next pair; normalize from the copy
                        o_sb = pB.tile([HD + 1, QB], f32, name="o_sb",
                                       tag="osb", bufs=3)
                        nc.vector.tensor_copy(o_sb, ps_o)
                        drow = pB.tile([1, QB], f32, name="drow", tag="drow",
                                       bufs=2)
                        nc.vector.tensor_scalar_add(drow, o_sb[HD:HD + 1, :],
                                                    POL_EPS)
                        rrow = pB.tile([1, QB], f32, name="rrow", tag="rrow",
                                       bufs=2)
                        nc.scalar.activation(rrow, drow, AF.Ln)
                        nc.scalar.activation(rrow, rrow, AF.Exp, scale=-1.0)
                        bcd = pB.tile([64, QB], f32, name="bcd", tag="bcd",
                                      bufs=2)
                        nc.gpsimd.partition_broadcast(bcd, rrow)
                        nc.vector.tensor_tensor(out=OTp[jp][hh:hh + 64, :],
                                                in0=o_sb[0:HD, :], in1=bcd,
                                                op=OP.mult)
                # proj + residual inside the attention scope: projw/x_own are
                # prefetched during phase B and the proj PSUM tag is part of
                # psB, so the PE rolls from the last O-matmul straight into
                # proj with no pool-handover wait.
                for st in range(4):
                    for (c0, csz) in [(0, 512), (512, 256)]:
                        ps_pr = psB.tile([128, 512], f32, name="ps_pr",
                                         tag="pspr", bufs=1)
                        for fj in range(FT):
                            nc.tensor.matmul(
                                ps_pr[:, 0:csz],
                                OTp[fj][:, st * 128:(st + 1) * 128],
                                projw[fj][:, c0:c0 + csz],
                                start=(fj == 0), stop=(fj == FT - 1),
                                skip_group_check=True)
                        nc.vector.tensor_tensor(
                            out=x_res[st][:, c0:c0 + csz], in0=ps_pr[:, 0:csz],
                            in1=x_own[st][:, c0:c0 + csz], op=OP.add)

        # --------------- phase C: proj + LN2 + MLP --------------------------
        with tc.tile_pool(name="pC", bufs=1) as pC:
            if not ln2_triv:
                ln2gb_sb = pC.tile([2, C], f32, name="ln2gb_sb")
                nc.sync.dma_start(out=ln2gb_sb, in_=ln2gb_d.ap())
                ln2g_bc = pC.tile([128, C], f32, name="ln2g_bc")
                nc.gpsimd.partition_broadcast(ln2g_bc, ln2gb_sb[0:1, :])
                ln2b_bc = pC.tile([128, C], f32, name="ln2b_bc")
                nc.gpsimd.partition_broadcast(ln2b_bc, ln2gb_sb[1:2, :])
            if not projb_triv:
                projb_sb = pC.tile([1, C], f32, name="projb_sb")
                nc.sync.dma_start(out=projb_sb, in_=projb_d.ap())
                projb_bc = pC.tile([128, C], f32, name="projb_bc")
                nc.gpsimd.partition_broadcast(projb_bc, projb_sb)
            if not fc2b_triv:
                fc2b_sb = pC.tile([1, C], f32, name="fc2b_sb")
                nc.sync.dma_start(out=fc2b_sb, in_=fc2b_d.ap())
                fc2b_bc = pC.tile([128, C], f32, name="fc2b_bc")
                nc.gpsimd.partition_broadcast(fc2b_bc, fc2b_sb)
            eps_col = pC.tile([128, 1], f32, name="eps_col")
            nc.vector.memset(eps_col, LN_EPS)
            if not projb_triv:
                for st in range(4):
                    nc.vector.tensor_add(x_res[st], x_res[st], projb_bc)
            h2T = [pC.tile([128, QB], f32r, name=f"h2t{cj}") for cj in range(CT)]

            with tc.tile_pool(name="psC1", bufs=1, space="PSUM") as psC1:
                h2s = []
                for st in range(4):
                    # LN2 (bn_stats over free dim, subgroups of 256)
                    stats = pC.tile([128, 3, 6], f32, name="stats",
                                    tag="stats", bufs=2)
                    for g in range(3):
                        nc.vector.bn_stats(
                            out=stats[:, g, :],
                            in_=x_res[st][:, g * 256:(g + 1) * 256])
                    mv = pC.tile([128, 2], f32, name="mv", tag="mv", bufs=2)
                    nc.vector.bn_aggr(out=mv, in_=stats)
                    ve2 = pC.tile([128, 1], f32, name="ve2", tag="ve2", bufs=2)
                    nc.vector.tensor_scalar_add(ve2, mv[:, 1:2], LN_EPS)
                    rs2 = pC.tile([128, 1], f32, name="rs2", tag="rs2", bufs=2)
                    nc.scalar.activation(rs2, ve2, AF.Ln)
                    nc.scalar.activation(rs2, rs2, AF.Exp, scale=-0.5)
                    h2 = pC.tile([128, C], f32, name="h2", tag="h2", bufs=4)
                    nc.vector.tensor_scalar(h2, x_res[st], mv[:, 0:1], rs2,
                                            op0=OP.subtract, op1=OP.mult)
                    if not ln2_triv:
                        nc.vector.tensor_mul(h2, h2, ln2g_bc)
                        nc.vector.tensor_add(h2, h2, ln2b_bc)
                    h2s.append(h2)
                # transpose h2 -> h2T (after all proj matmuls so the PE can
                # run proj(st+1) while LN2(st) computes on the DVE)
                for st in range(4):
                    for cj in range(CT):
                        ps_tr = psC1.tile([128, 128], f32, name="ps_tr",
                                          tag="pstr", bufs=3)
                        nc.tensor.transpose(
                            ps_tr, h2s[st][:, cj * 128:(cj + 1) * 128], eye_sb)
                        nc.vector.tensor_copy(
                            h2T[cj][:, st * 128:(st + 1) * 128], ps_tr)
                # fc1 + gelu -> gT
                gT = [pC.tile([128, QB], f32r, name=f"gt{hj}")
                      for hj in range(HB)]
                for hblk in range(6):
                    w1 = []
                    for cj in range(CT):
                        w1t = pC.tile([128, 512], f32r, name="w1",
                                      tag=f"w1_{cj}", bufs=2)
                        nc.sync.dma_start(out=w1t, in_=fc1w_d.ap()[cj, hblk])
                        w1.append(w1t)
                    for hl_ in range(4):
                        hj = hblk * 4 + hl_
                        ps_f1 = psC1.tile([128, QB], f32, name="ps_f1",
                                          tag="psf1", bufs=3)
                        for cj in range(CT):
                            nc.tensor.matmul(
                                ps_f1, w1[cj][:, hl_ * 128:(hl_ + 1) * 128],
                                h2T[cj], start=(cj == 0), stop=(cj == CT - 1))
                        nc.scalar.activation(gT[hj], ps_f1, AF.Gelu,
                                             bias=fc1b_sb[:, hj:hj + 1])

            # fc2 + residual + output
            with tc.tile_pool(name="psC2", bufs=1, space="PSUM") as psC2:
                ps_f2 = [psC2.tile([128, C], f32, name=f"psf2_{st}",
                                   tag=f"psf2_{st}", bufs=1)
                         for st in range(4)]
                for hj in range(HB):
                    w2 = pC.tile([128, C], f32r, name="w2", tag="w2", bufs=4)
                    nc.gpsimd.dma_start(
                        out=w2, in_=fc2w_d.ap()[hj * 128:(hj + 1) * 128, :])
                    for st in range(4):
                        for (c0, csz) in [(0, 512), (512, 256)]:
                            nc.tensor.matmul(
                                ps_f2[st][:, c0:c0 + csz],
                                gT[hj][:, st * 128:(st + 1) * 128],
                                w2[:, c0:c0 + csz],
                                start=(hj == 0), stop=(hj == HB - 1),
                                skip_group_check=True)
                for st in range(4):
                    out_t = pC.tile([128, C], f32, name="out_t", tag="outt",
                                    bufs=2)
                    nc.vector.tensor_tensor(out=out_t, in0=ps_f2[st],
                                            in1=x_res[st], op=OP.add)
                    if not fc2b_triv:
                        nc.vector.tensor_add(out_t, out_t, fc2b_bc)
                    nc.sync.dma_start(
                        out=yout_d.ap()[st * 128:(st + 1) * 128, :],
                        in_=out_t)

    # Prefer the combined natural_log_exp table set so the Ln/Exp mix in this
    # kernel resolves to ONE ACT table set (the default chooser picks
    # single-anchor sets and thrashes ~1.3us per switch).
    import concourse.bacc as _bacc_mod
    _orig_tables = _bacc_mod.get_activation_tables

    def _pref_tables(arch):
        # act_func_set_id is positional, so keep order/length; just hide
        # Exp/Ln from every other set so both resolve to the combined one.
        t = _orig_tables(arch)
        out = {}
        for name, fns in t.items():
            if name != "natural_log_exp_and_others":
                fns = {f for f in fns if f not in (AF.Exp, AF.Ln)}
            out[name] = set(fns)
        return out

    _bacc_mod.get_activation_tables = _pref_tables
    try:
        nc.compile()
    finally:
        _bacc_mod.get_activation_tables = _orig_tables
    return nc


def _prep_shared(qkv_w, proj_w, fc1_w, fc2_w, fc1_b):
    """Host-side weight packing (shared across all cores)."""
    qkvT = np.ascontiguousarray(qkv_w.T)          # [C, 3C]: q | k | v
    wq = qkvT[:, 0:C]
    wk = qkvT[:, C:2 * C]
    wv = qkvT[:, 2 * C:3 * C]

    def pack_T(w):
        # [C, F] -> [FT, 128, CT*128]; [fj, p, ci*128+f] = w[ci*128+p, fj*128+f]
        t = w.reshape(CT, 128, FT, 128)
        return np.ascontiguousarray(
            t.transpose(2, 1, 0, 3).reshape(FT, 128, CT * 128))

    wq_packT = pack_T(wq)
    wk_packT = pack_T(wk)
    # wv: [128, ci*C + f] = wv[ci*128+p, f]
    wv_packT = np.ascontiguousarray(
        wv.reshape(CT, 128, C).transpose(1, 0, 2).reshape(128, CT * C))
    projwT = np.ascontiguousarray(proj_w.T)       # [F, C]
    fc1T = np.ascontiguousarray(fc1_w.T)          # [C, HID]
    fc1_pack = np.empty((CT, 6, 128, 512), np.float32)
    for cj in range(CT):
        for hblk in range(6):
            fc1_pack[cj, hblk] = fc1T[cj * 128:(cj + 1) * 128,
                                      hblk * 512:(hblk + 1) * 512]
    fc2T = np.ascontiguousarray(fc2_w.T)          # [HID, C]
    fc1b_cols = np.ascontiguousarray(fc1_b.reshape(HB, 128).T)
    eye = np.eye(128, dtype=np.float32)
    return dict(wq_packT=wq_packT, wk_packT=wk_packT, wv_packT=wv_packT,
                projwT=projwT, fc1w_pack=fc1_pack, fc2wT=fc2T,
                fc1b=fc1b_cols, eye=eye)


def kernel(x, policy, ln1_g, ln1_b, qkv_w, proj_w, proj_b, ln2_g, ln2_b,
           fc1_w, fc1_b, fc2_w, fc2_b):
    global LAST_RESULTS
    x = np.asarray(x, np.float32)
    policy = np.asarray(policy, np.float32)

    ln1_triv = bool(np.all(ln1_g == 1.0) and np.all(ln1_b == 0.0))
    ln2_triv = bool(np.all(ln2_g == 1.0) and np.all(ln2_b == 0.0))
    projb_triv = bool(np.all(proj_b == 0.0))
    fc2b_triv = bool(np.all(fc2_b == 0.0))
    # key compaction: each core keeps its own 512 queries as keys 0:512 plus
    # all unmasked other keys; masked non-own keys never attend anywhere
    # (their post-mask P is ~e-50) so they are dropped from K/V entirely.
    pol2 = policy[:, :, 0] > 0.5
    cols_per_core = []
    for c in range(NCORES):
        b_, qoff = c // 4, (c % 4) * QB
        own = np.arange(qoff, qoff + QB)
        other = np.concatenate([np.arange(0, qoff), np.arange(qoff + QB, N)])
        other = other[pol2[b_, other]]
        cols_per_core.append(np.concatenate([own, other]))
    kmax = max(len(cl) for cl in cols_per_core)
    kpad = ((kmax + 511) // 512) * 512

    key = (ln1_triv, ln2_triv, projb_triv, fc2b_triv, kpad)
    if key not in _prog_cache:
        _prog_cache[key] = _build_program(*key)
    nc = _prog_cache[key]
    kt_n = kpad // 128

    shared = _prep_shared(np.asarray(qkv_w, np.float32),
                          np.asarray(proj_w, np.float32),
                          np.asarray(fc1_w, np.float32),
                          np.asarray(fc2_w, np.float32),
                          np.asarray(fc1_b, np.float32))
    if not ln1_triv:
        g = np.asarray(ln1_g, np.float32).reshape(CT, 128).T
        b = np.asarray(ln1_b, np.float32).reshape(CT, 128).T
        shared["ln1gb"] = np.ascontiguousarray(np.concatenate([g, b], axis=1))
    if not ln2_triv:
        shared["ln2gb"] = np.ascontiguousarray(
            np.stack([np.asarray(ln2_g, np.float32),
                      np.asarray(ln2_b, np.float32)]))
    if not projb_triv:
        shared["projb"] = np.asarray(proj_b, np.float32).reshape(1, C)
    if not fc2b_triv:
        shared["fc2b"] = np.asarray(fc2_b, np.float32).reshape(1, C)

    in_maps = []
    for c in range(NCORES):
        b_, qoff = c // 4, (c % 4) * QB
        cols = cols_per_core[c]
        xT_c = np.zeros((C, kpad), np.float32)
        xT_c[:, :len(cols)] = x[b_].T[:, cols]
        polp = np.zeros(kpad, np.float32)
        polp[:len(cols)] = policy[b_, cols, 0]
        pol_cols = np.ascontiguousarray(polp.reshape(kt_n, 128).T)
        lnp_cols = np.ascontiguousarray(
            np.where(polp > 0.5, 0.0, MASK_NEG).astype(np.float32)
            .reshape(kt_n, 128).T)
        m = dict(shared)
        m["xT"] = xT_c
        m["x_own"] = np.ascontiguousarray(x[b_, qoff:qoff + QB])
        m["pol"] = pol_cols
        m["lnp"] = lnp_cols
        in_maps.append(m)

    res = run_bass_kernel_spmd(nc, in_maps, core_ids=list(range(NCORES)),
                               trace=TRACE, **TRACE_KWARGS)
    LAST_RESULTS = res
    out = np.empty((B, N, C), np.float32)
    for c in range(NCORES):
        b_, qoff = c // 4, (c % 4) * QB
        out[b_, qoff:qoff + QB] = res.results[c]["yout"]
    return out


# revision 25
# speedup vs baseline: 1.0599x; 1.0599x over previous
"""Trainium2 Bass kernel for nn_Block_22497038696617 (dense transformer block).

Block: pre-LN attention with policy-masked softmax + pre-LN MLP (exact GELU).
  B=2, N=2048, C=768, H=12 heads x 64, HID=3072, fp32 in/out.

Sharding (8 cores, zero cross-core communication, single SPMD launch):
  core c -> batch b = c//4, query block qoff = (c%4)*512.
  Each core computes LN1 + K/V for the full sequence of its batch
  (replicated across the 4 cores of that batch), Q/attention/proj/MLP for
  its own 512 query rows, and writes its [512, 768] output slice.
  Host gathers the 8 slices into the full [2, 2048, 768] output.

Key compaction + program-uniformity trick: attention is permutation-invariant
over keys, and a key whose policy bit is 0 contributes ~0 to every query (its
post-mask P is ~e-50) EXCEPT to its own query via the "always attend to self"
diagonal. So each core's key axis is rebuilt on the host as
  [its own 512 queries (always kept), all unmasked other keys, zero padding]
padded to a multiple of 512 (typically 1536 of the original 2048). This (a)
drops ~25% of the K/V/attention work, and (b) pins the diagonal exception to
k-tiles 0..3 at column offset t*128, making the SPMD program identical on all
cores even though each core has a different query block.

Matmuls run in float32r (TF32-like, ~1.5e-4 rel err, full PE speed at
free-dim >= 256) with fp32 PSUM accumulation. Softmax skips max-subtraction
(logits are O(1); fp32 exp is safe) and folds the policy mask into the exp as
a per-key bias of ln(policy) (0 or -50). The softmax denominator comes free
from a ones-column appended to V. The numerator +POL_EPS/n term (~5e-10
absolute on O(1e-3) values) is dropped as negligible; denominator +POL_EPS
is kept. Attention layout is transposed (S^T [keys, queries]) so softmax
masking is per-partition and no P transposes are needed. Heads are processed
in pairs sharing one [128, 1024] PSUM tile so each k-tile needs a single exp
instruction, and the two S-matmuls land in disjoint PE row groups (rows 0-63 /
64-127) and can execute concurrently.
"""

from contextlib import ExitStack

import numpy as np

import concourse.bacc as bacc
import concourse.mybir as mybir
import concourse.tile as tile
from concourse.bass_utils import run_bass_kernel_spmd

f32 = mybir.dt.float32
f32r = mybir.dt.float32r
AF = mybir.ActivationFunctionType
OP = mybir.AluOpType

B, N, C = 2, 2048, 768
H, HD = 12, 64
HID = 3072
NCORES = 8
QB = 512                 # own query rows per core
CT = C // 128            # 6 c-tiles
FT = C // 128            # 6 f-tiles (H*HD == C)
KTN = N // 128           # 16 k-tiles
HB = HID // 128          # 24 hid-tiles
SCALE = HD ** -0.5
LN_EPS = 1e-5
POL_EPS = 1e-6
MASK_NEG = -50.0

TRACE = False            # set True by the dev harness for profiling runs
TRACE_KWARGS = {}
LAST_RESULTS = None      # BassKernelResults of the last run (for timing)

_prog_cache = {}


def _build_program(ln1_triv, ln2_triv, projb_triv, fc2b_triv, kpad):
    kt_n = kpad // 128          # k-tiles after key compaction
    kq_n = kpad // 512          # 512-wide key chunks
    nc = bacc.Bacc("TRN2", target_bir_lowering=False, debug=False,
                   num_devices=NCORES)

    # ---- DRAM I/O (f32r dtype for tensors DMA'd straight into matmuls) ----
    xT_d = nc.dram_tensor("xT", [C, kpad], f32r, kind="ExternalInput")
    xown_d = nc.dram_tensor("x_own", [QB, C], f32, kind="ExternalInput")
    pol_d = nc.dram_tensor("pol", [128, kt_n], f32, kind="ExternalInput")
    lnp_d = nc.dram_tensor("lnp", [128, kt_n], f32, kind="ExternalInput")
    # weight packs: [p, ci, f] layout so one DMA loads a [128, ...] tile whose
    # [:, ci, :] slice is the lhsT/rhs tile for c-tile ci (contiguous lines)
    wq_d = nc.dram_tensor("wq_packT", [FT, 128, CT * 128], f32r,
                          kind="ExternalInput")
    wk_d = nc.dram_tensor("wk_packT", [FT, 128, CT * 128], f32r,
                          kind="ExternalInput")
    wv_d = nc.dram_tensor("wv_packT", [128, CT * C], f32r,
                          kind="ExternalInput")
    projw_d = nc.dram_tensor("projwT", [C, C], f32r, kind="ExternalInput")
    fc1w_d = nc.dram_tensor("fc1w_pack", [CT, 6, 128, 512], f32r,
                            kind="ExternalInput")
    fc2w_d = nc.dram_tensor("fc2wT", [HID, C], f32r, kind="ExternalInput")
    fc1b_d = nc.dram_tensor("fc1b", [128, HB], f32, kind="ExternalInput")
    eye_d = nc.dram_tensor("eye", [128, 128], f32, kind="ExternalInput")
    if not ln1_triv:
        ln1gb_d = nc.dram_tensor("ln1gb", [128, 2 * CT], f32,
                                 kind="ExternalInput")
    if not ln2_triv:
        ln2gb_d = nc.dram_tensor("ln2gb", [2, C], f32, kind="ExternalInput")
    if not projb_triv:
        projb_d = nc.dram_tensor("projb", [1, C], f32, kind="ExternalInput")
    if not fc2b_triv:
        fc2b_d = nc.dram_tensor("fc2b", [1, C], f32, kind="ExternalInput")
    yout_d = nc.dram_tensor("yout", [QB, C], f32, kind="ExternalOutput")

    with tile.TileContext(nc) as tc, ExitStack() as ctx:
        # ---------------- constants + whole-kernel persistents --------------
        pG = ctx.enter_context(tc.tile_pool(name="pG", bufs=1))
        eye_sb = pG.tile([128, 128], f32, name="eye_sb")
        nc.gpsimd.dma_start(out=eye_sb, in_=eye_d.ap())
        pol_sb = pG.tile([128, kt_n], f32, name="pol_sb")
        nc.gpsimd.dma_start(out=pol_sb, in_=pol_d.ap())
        lnp_sb = pG.tile([128, kt_n], f32, name="lnp_sb")
        nc.gpsimd.dma_start(out=lnp_sb, in_=lnp_d.ap())
        fc1b_sb = pG.tile([128, HB], f32, name="fc1b_sb")
        nc.gpsimd.dma_start(out=fc1b_sb, in_=fc1b_d.ap())
        invpol_sb = pG.tile([128, kt_n], f32, name="invpol_sb")
        nc.vector.tensor_scalar(invpol_sb, pol_sb, -1.0, 1.0,
                                op0=OP.mult, op1=OP.add)
        ones_col = pG.tile([128, 1], f32r, name="ones_col")
        nc.vector.memset(ones_col.bitcast(f32), 1.0)
        # attention output, transposed, per head-pair: OTp[j] rows = features
        # of heads (2j, 2j+1), cols = own queries
        OTp = [pG.tile([128, QB], f32r, name=f"otp{j}") for j in range(FT)]
        # attention-residual rows live here so proj (emitted inside the
        # attention scope) can write them and phase C can read them
        x_res = [pG.tile([128, C], f32, name=f"xres{st}") for st in range(4)]

        # ======================= phase A + B scope ==========================
        with tc.tile_pool(name="pAB", bufs=1) as pAB:
            KTp = [pAB.tile([128, kpad], f32r, name=f"ktp{j}") for j in range(FT)]
            QTp = [pAB.tile([128, QB], f32r, name=f"qtp{j}") for j in range(FT)]
            vpad = [pAB.tile([128, H, HD + 1], f32r, name=f"vpad{t}")
                    for t in range(kt_n)]

            # --------------- phase A: LN1 + QKV projections -----------------
            # Software-pipelined: quarter q's LN stats/apply overlap quarter
            # q-1's K/V/Q matmuls.
            with tc.tile_pool(name="pA", bufs=1) as pA, \
                 tc.tile_pool(name="psA", bufs=1, space="PSUM") as psA:
                if not ln1_triv:
                    ln1gb_sb = pA.tile([128, 2 * CT], f32, name="ln1gb_sb")
                    nc.sync.dma_start(out=ln1gb_sb, in_=ln1gb_d.ap())
                # resident V weights: [p, ci, f] single contiguous DMA
                wv_sb = pA.tile([128, CT, C], f32r, name="wv_sb")
                nc.gpsimd.dma_start(
                    out=wv_sb.rearrange("p a b -> p (a b)"), in_=wv_d.ap())

                def ln_loads_stats(qr):
                    """x.T loads + stats matmuls for one key chunk."""
                    s0 = qr * 512
                    xt = []
                    for ci in range(CT):
                        t_ = pA.tile([128, 512], f32r, name="xt", tag="xt",
                                     bufs=6)
                        nc.sync.dma_start(
                            out=t_,
                            in_=xT_d.ap()[ci * 128:(ci + 1) * 128, s0:s0 + 512])
                        xt.append(t_)
                    # stats via ones-matmuls (sum over c = partition dim)
                    ps_mean = psA.tile([1, 512], f32, name="ps_mean",
                                       tag="psmean", bufs=2)
                    ps_sq = psA.tile([1, 512], f32, name="ps_sq",
                                     tag="pssq", bufs=2)
                    for ci in range(CT):
                        nc.tensor.matmul(ps_mean, ones_col, xt[ci],
                                         start=(ci == 0), stop=(ci == CT - 1))
                    for ci in range(CT):
                        xsq = pA.tile([128, 512], f32r, name="xsq", tag="d_t",
                                      bufs=2)
                        nc.vector.tensor_mul(xsq, xt[ci].bitcast(f32),
                                             xt[ci].bitcast(f32))
                        nc.tensor.matmul(ps_sq, ones_col, xsq,
                                         start=(ci == 0), stop=(ci == CT - 1))
                    return xt, ps_mean, ps_sq

                def ln_rows_hl(qr, stage):
                    """LN1 row stats -> broadcast -> h_ln.T build."""
                    xt, ps_mean, ps_sq = stage
                    # rows: mean, var+eps, rstd = exp(-0.5*ln(var+eps))
                    def row(nm):
                        return pA.tile([1, 512], f32, name=nm, tag="rows",
                                       bufs=3)
                    mrow = row("mrow")
                    nc.vector.tensor_scalar_mul(mrow, ps_mean, 1.0 / C)
                    ve = row("ve")
                    nc.vector.tensor_scalar(ve, ps_sq, 1.0 / C, LN_EPS,
                                            op0=OP.mult, op1=OP.add)
                    m2 = row("m2")
                    nc.vector.tensor_mul(m2, mrow, mrow)
                    nc.vector.tensor_sub(ve, ve, m2)
                    nc.scalar.activation(ve, ve, AF.Ln)
                    r0 = row("r0")
                    nc.scalar.activation(r0, ve, AF.Exp, scale=-0.5)
                    bc_m = pA.tile([128, 512], f32, name="bc_m", tag="bc_m",
                                   bufs=1)
                    nc.gpsimd.partition_broadcast(bc_m, mrow)
                    bc_r = pA.tile([128, 512], f32, name="bc_r", tag="bc_r",
                                   bufs=1)
                    nc.gpsimd.partition_broadcast(bc_r, r0)
                    hl = []
                    for ci in range(CT):
                        d_t = pA.tile([128, 512], f32, name="d_t", tag="d_t",
                                      bufs=2)
                        nc.vector.tensor_sub(d_t, xt[ci].bitcast(f32), bc_m)
                        h_ = pA.tile([128, 512], f32r, name="hl", tag="hl",
                                     bufs=12)
                        if ln1_triv:
                            nc.vector.tensor_tensor(out=h_, in0=d_t, in1=bc_r,
                                                    op=OP.mult)
                        else:
                            nc.vector.tensor_tensor(out=d_t, in0=d_t, in1=bc_r,
                                                    op=OP.mult)
                            nc.vector.tensor_scalar(
                                h_, d_t, ln1gb_sb[:, ci:ci + 1],
                                ln1gb_sb[:, CT + ci:CT + ci + 1],
                                op0=OP.mult, op1=OP.add)
                        hl.append(h_)
                    return hl

                def kvq_stage(qr, hl):
                    """K/V (+Q for quarter 0) matmuls for one quarter."""
                    s0 = qr * 512
                    for fj in range(FT):
                        wk = pA.tile([128, CT, 128], f32r, name="wk", tag="wk",
                                     bufs=2)
                        nc.gpsimd.dma_start(
                            out=wk.rearrange("p a b -> p (a b)"),
                            in_=wk_d.ap()[fj])
                        psk = psA.tile([128, 512], f32, name="psk",
                                       tag="pskv", bufs=3)
                        for ci in range(CT):
                            nc.tensor.matmul(psk, wk[:, ci, :], hl[ci],
                                             start=(ci == 0),
                                             stop=(ci == CT - 1))
                        nc.vector.tensor_copy(KTp[fj][:, s0:s0 + 512], psk)
                    for si in range(4):
                        st = qr * 4 + si
                        for fc in range(2):
                            f0 = fc * 512
                            wsz = 512 if fc == 0 else 256
                            psv = psA.tile([128, 512], f32, name="psv",
                                           tag="pskv", bufs=3)
                            for ci in range(CT):
                                nc.tensor.matmul(
                                    psv[:, 0:wsz],
                                    hl[ci][:, si * 128:(si + 1) * 128],
                                    wv_sb[:, ci, f0:f0 + wsz],
                                    start=(ci == 0), stop=(ci == CT - 1))
                            nh = wsz // HD
                            h0 = 0 if fc == 0 else 8
                            nc.vector.tensor_copy(
                                vpad[st][:, h0:h0 + nh, 0:HD],
                                psv[:, 0:wsz].rearrange(
                                    "p (h d) -> p h d", h=nh))
                        nc.vector.memset(vpad[st].bitcast(f32)[:, :, HD], 1.0)
                    if qr == 0:
                        # own queries are keys 0:512 => Q.T from quarter 0
                        for fj in range(FT):
                            wq = pA.tile([128, CT, 128], f32r, name="wq",
                                         tag="wk", bufs=2)
                            nc.gpsimd.dma_start(
                                out=wq.rearrange("p a b -> p (a b)"),
                                in_=wq_d.ap()[fj])
                            psq = psA.tile([128, 512], f32, name="psq",
                                           tag="pskv", bufs=3)
                            for ci in range(CT):
                                nc.tensor.matmul(psq, wq[:, ci, :], hl[ci],
                                                 start=(ci == 0),
                                                 stop=(ci == CT - 1))
                            nc.vector.tensor_copy(QTp[fj], psq)

                # 2-deep software pipeline: PE order is
                #   stats(0), stats(1), kvq(0), stats(2), kvq(1), ...
                # so chunk q+1's stats matmuls (gated on DMA) issue while
                # chunk q's K/V inputs are already on-chip, and the LN row /
                # h_ln.T DVE work of q+1 overlaps kvq(q) on the PE.
                stage = ln_loads_stats(0)
                hl_prev = ln_rows_hl(0, stage)
                for qr in range(1, kq_n):
                    stage = ln_loads_stats(qr)
                    kvq_stage(qr - 1, hl_prev)
                    hl_prev = ln_rows_hl(qr, stage)
                kvq_stage(kq_n - 1, hl_prev)

            # --------------- phase B: attention (head pairs) ----------------
            with tc.tile_pool(name="pB", bufs=1) as pB, \
                 tc.tile_pool(name="psB", bufs=1, space="PSUM") as psB:
                projw = [pB.tile([128, C], f32r, name=f"pjw{fj}")
                         for fj in range(FT)]
                for fj in range(FT):
                    nc.sync.dma_start(
                        out=projw[fj],
                        in_=projw_d.ap()[fj * 128:(fj + 1) * 128, :])
                x_own = [pB.tile([128, C], f32, name=f"xown{st}")
                         for st in range(4)]
                for st in range(4):
                    nc.sync.dma_start(
                        out=x_own[st],
                        in_=xown_d.ap()[st * 128:(st + 1) * 128, :])
                for jp in range(H // 2):
                    h0, h1 = 2 * jp, 2 * jp + 1
                    ps_o0 = psB.tile([HD + 1, QB], f32, name="ps_o0",
                                     tag="pso", bufs=2)
                    ps_o1 = psB.tile([HD + 1, QB], f32, name="ps_o1",
                                     tag="pso", bufs=2)

                    def o_mms(ti, t, p_t):
                        nc.tensor.matmul(ps_o0, vpad[t][:, h0, :],
                                         p_t[:, 0:QB],
                                         start=(ti == 0), stop=(ti == kt_n - 1),
                                         skip_group_check=True)
                        nc.tensor.matmul(ps_o1, vpad[t][:, h1, :],
                                         p_t[:, QB:2 * QB],
                                         start=(ti == 0), stop=(ti == kt_n - 1),
                                         skip_group_check=True)

                    prev = None
                    # diag tiles (0..3) carry extra DVE mask work; process
                    # them LAST so a pair's first O-matmuls aren't gated on
                    # the DVE chain right at the pair boundary
                    t_order = list(range(4, kt_n)) + [0, 1, 2, 3]
                    for ti, t in enumerate(t_order):
                        ps_s = psB.tile([128, 2 * QB], f32, name="ps_s",
                                        tag="pss", bufs=2)
                        nc.tensor.matmul(
                            ps_s[:, 0:QB],
                            KTp[jp][0:64, t * 128:(t + 1) * 128],
                            QTp[jp][0:64, :],
                            start=True, stop=True, skip_group_check=True)
                        nc.tensor.matmul(
                            ps_s[:, QB:2 * QB],
                            KTp[jp][64:128, t * 128:(t + 1) * 128],
                            QTp[jp][64:128, :],
                            start=True, stop=True, skip_group_check=True)
                        p_t = pB.tile([128, 2 * QB], f32r, name="p_t",
                                      tag="pt", bufs=6)
                        if t >= 4:
                            # mask folded into exp: exp(scale*s + ln(policy))
                            nc.scalar.activation(p_t, ps_s, AF.Exp,
                                                 bias=lnp_sb[:, t:t + 1],
                                                 scale=SCALE)
                        else:
                            # diagonal k-tile: need exp * max(eye, policy).
                            # p = exp(s); dc = p*eye*(1-pol) on the two diag
                            # blocks; p *= pol; p += dc.
                            off = t * 128
                            nc.scalar.activation(p_t, ps_s, AF.Exp,
                                                 scale=SCALE)
                            cor = pB.tile([128, 128], f32, name="cor",
                                          tag="cor", bufs=2)
                            nc.vector.tensor_scalar_mul(
                                cor, eye_sb, invpol_sb[:, t:t + 1])
                            dc0 = pB.tile([128, 128], f32, name="dc0",
                                          tag="dc0", bufs=2)
                            nc.vector.tensor_tensor(
                                out=dc0, in0=p_t[:, off:off + 128].bitcast(f32),
                                in1=cor, op=OP.mult)
                            dc1 = pB.tile([128, 128], f32, name="dc1",
                                          tag="dc1", bufs=2)
                            nc.vector.tensor_tensor(
                                out=dc1,
                                in0=p_t[:, QB + off:QB + off + 128].bitcast(f32),
                                in1=cor, op=OP.mult)
                            nc.vector.tensor_scalar_mul(
                                p_t, p_t.bitcast(f32), pol_sb[:, t:t + 1])
                            nc.vector.tensor_tensor(
                                out=p_t[:, off:off + 128],
                                in0=p_t[:, off:off + 128].bitcast(f32),
                                in1=dc0, op=OP.add)
                            nc.vector.tensor_tensor(
                                out=p_t[:, QB + off:QB + off + 128],
                                in0=p_t[:, QB + off:QB + off + 128].bitcast(f32),
                                in1=dc1, op=OP.add)
                        # software pipeline: O matmuls trail by one k-tile so
                        # the PE can run S(t+1) while the ACT exp(t) finishes
                        if prev is not None:
                            o_mms(*prev)
                        prev = (ti, t, p_t)
                    o_mms(*prev)
                    # normalize: O / (denominator + POL_EPS); the reciprocal
                    # runs on ACT as exp(-ln(d)) so the in-order DVE queue
                    # isn't blocked ahead of the next pair's mask ops
                    for hh, ps_o in ((0, ps_o0), (64, ps_o1)):
                        # copy PSUM out immediately so the accumulator bank is
                        # released for the next pair; normalize from the copy
                        o_sb = pB.tile([HD + 1, QB], f32, name="o_sb",
                                       tag="osb", bufs=3)
                        nc.vector.tensor_copy(o_sb, ps_o)
                        drow = pB.tile([1, QB], f32, name="drow", tag="drow",
                                       bufs=2)
                        nc.vector.tensor_scalar_add(drow, o_sb[HD:HD + 1, :],
                                                    POL_EPS)
                        rrow = pB.tile([1, QB], f32, name="rrow", tag="rrow",
                                       bufs=2)
                        nc.scalar.activation(rrow, drow, AF.Ln)
                        nc.scalar.activation(rrow, rrow, AF.Exp, scale=-1.0)
                        bcd = pB.tile([64, QB], f32, name="bcd", tag="bcd",
                                      bufs=2)
                        nc.gpsimd.partition_broadcast(bcd, rrow)
                        nc.vector.tensor_tensor(out=OTp[jp][hh:hh + 64, :],
                                                in0=o_sb[0:HD, :], in1=bcd,
                                                op=OP.mult)
                # proj + residual inside the attention scope: projw/x_own are
                # prefetched during phase B and the proj PSUM tag is part of
                # psB, so the PE rolls from the last O-matmul straight into
                # proj with no pool-handover wait.
                for st in range(4):
                    ps_pr = psB.tile([128, C], f32, name="ps_pr", tag="pspr",
                                     bufs=1)
                    for (c0, csz) in [(0, 512), (512, 256)]:
                        for fj in range(FT):
                            nc.tensor.matmul(
                                ps_pr[:, c0:c0 + csz],
                                OTp[fj][:, st * 128:(st + 1) * 128],
                                projw[fj][:, c0:c0 + csz],
                                start=(fj == 0), stop=(fj == FT - 1),
                                skip_group_check=True)
                    nc.vector.tensor_tensor(out=x_res[st], in0=ps_pr,
                                            in1=x_own[st], op=OP.add)

        # --------------- phase C: proj + LN2 + MLP --------------------------
        with tc.tile_pool(name="pC", bufs=1) as pC:
            if not ln2_triv:
                ln2gb_sb = pC.tile([2, C], f32, name="ln2gb_sb")
                nc.sync.dma_start(out=ln2gb_sb, in_=ln2gb_d.ap())
                ln2g_bc = pC.tile([128, C], f32, name="ln2g_bc")
                nc.gpsimd.partition_broadcast(ln2g_bc, ln2gb_sb[0:1, :])
                ln2b_bc = pC.tile([128, C], f32, name="ln2b_bc")
                nc.gpsimd.partition_broadcast(ln2b_bc, ln2gb_sb[1:2, :])
            if not projb_triv:
                projb_sb = pC.tile([1, C], f32, name="projb_sb")
                nc.sync.dma_start(out=projb_sb, in_=projb_d.ap())
                projb_bc = pC.tile([128, C], f32, name="projb_bc")
                nc.gpsimd.partition_broadcast(projb_bc, projb_sb)
            if not fc2b_triv:
                fc2b_sb = pC.tile([1, C], f32, name="fc2b_sb")
                nc.sync.dma_start(out=fc2b_sb, in_=fc2b_d.ap())
                fc2b_bc = pC.tile([128, C], f32, name="fc2b_bc")
                nc.gpsimd.partition_broadcast(fc2b_bc, fc2b_sb)
            eps_col = pC.tile([128, 1], f32, name="eps_col")
            nc.vector.memset(eps_col, LN_EPS)
            if not projb_triv:
                for st in range(4):
                    nc.vector.tensor_add(x_res[st], x_res[st], projb_bc)
            h2T = [pC.tile([128, QB], f32r, name=f"h2t{cj}") for cj in range(CT)]

            with tc.tile_pool(name="psC1", bufs=1, space="PSUM") as psC1:
                h2s = []
                for st in range(4):
                    # LN2 (bn_stats over free dim, subgroups of 256)
                    stats = pC.tile([128, 3, 6], f32, name="stats",
                                    tag="stats", bufs=2)
                    for g in range(3):
                        nc.vector.bn_stats(
                            out=stats[:, g, :],
                            in_=x_res[st][:, g * 256:(g + 1) * 256])
                    mv = pC.tile([128, 2], f32, name="mv", tag="mv", bufs=2)
                    nc.vector.bn_aggr(out=mv, in_=stats)
                    ve2 = pC.tile([128, 1], f32, name="ve2", tag="ve2", bufs=2)
                    nc.vector.tensor_scalar_add(ve2, mv[:, 1:2], LN_EPS)
                    rs2 = pC.tile([128, 1], f32, name="rs2", tag="rs2", bufs=2)
                    nc.scalar.activation(rs2, ve2, AF.Ln)
                    nc.scalar.activation(rs2, rs2, AF.Exp, scale=-0.5)
                    h2 = pC.tile([128, C], f32, name="h2", tag="h2", bufs=4)
                    nc.vector.tensor_scalar(h2, x_res[st], mv[:, 0:1], rs2,
                                            op0=OP.subtract, op1=OP.mult)
                    if not ln2_triv:
                        nc.vector.tensor_mul(h2, h2, ln2g_bc)
                        nc.vector.tensor_add(h2, h2, ln2b_bc)
                    h2s.append(h2)
                # transpose h2 -> h2T (after all proj matmuls so the PE can
                # run proj(st+1) while LN2(st) computes on the DVE)
                for st in range(4):
                    for cj in range(CT):
                        ps_tr = psC1.tile([128, 128], f32, name="ps_tr",
                                          tag="pstr", bufs=3)
                        nc.tensor.transpose(
                            ps_tr, h2s[st][:, cj * 128:(cj + 1) * 128], eye_sb)
                        nc.vector.tensor_copy(
                            h2T[cj][:, st * 128:(st + 1) * 128], ps_tr)
                # fc1 + gelu -> gT
                gT = [pC.tile([128, QB], f32r, name=f"gt{hj}")
                      for hj in range(HB)]
                for hblk in range(6):
                    w1 = []
                    for cj in range(CT):
                        w1t = pC.tile([128, 512], f32r, name="w1",
                                      tag=f"w1_{cj}", bufs=2)
                        nc.sync.dma_start(out=w1t, in_=fc1w_d.ap()[cj, hblk])
                        w1.append(w1t)
                    for hl_ in range(4):
                        hj = hblk * 4 + hl_
                        ps_f1 = psC1.tile([128, QB], f32, name="ps_f1",
                                          tag="psf1", bufs=3)
                        for cj in range(CT):
                            nc.tensor.matmul(
                                ps_f1, w1[cj][:, hl_ * 128:(hl_ + 1) * 128],
                                h2T[cj], start=(cj == 0), stop=(cj == CT - 1))
                        nc.scalar.activation(gT[hj], ps_f1, AF.Gelu,
                                             bias=fc1b_sb[:, hj:hj + 1])

            # fc2 + residual + output
            with tc.tile_pool(name="psC2", bufs=1, space="PSUM") as psC2:
                ps_f2 = [psC2.tile([128, C], f32, name=f"psf2_{st}",
                                   tag=f"psf2_{st}", bufs=1)
                         for st in range(4)]
                for hj in range(HB):
                    w2 = pC.tile([128, C], f32r, name="w2", tag="w2", bufs=4)
                    nc.gpsimd.dma_start(
                        out=w2, in_=fc2w_d.ap()[hj * 128:(hj + 1) * 128, :])
                    for st in range(4):
                        for (c0, csz) in [(0, 512), (512, 256)]:
                            nc.tensor.matmul(
                                ps_f2[st][:, c0:c0 + csz],
                                gT[hj][:, st * 128:(st + 1) * 128],
                                w2[:, c0:c0 + csz],
                                start=(hj == 0), stop=(hj == HB - 1),
                                skip_group_check=True)
                for st in range(4):
                    out_t = pC.tile([128, C], f32, name="out_t", tag="outt",
                                    bufs=2)
                    nc.vector.tensor_tensor(out=out_t, in0=ps_f2[st],
                                            in1=x_res[st], op=OP.add)
                    if not fc2b_triv:
                        nc.vector.tensor_add(out_t, out_t, fc2b_bc)
                    nc.sync.dma_start(
                        out=yout_d.ap()[st * 128:(st + 1) * 128, :],
                        in_=out_t)

    # Prefer the combined natural_log_exp table set so the Ln/Exp mix in this
    # kernel resolves to ONE ACT table set (the default chooser picks
    # single-anchor sets and thrashes ~1.3us per switch).
    import concourse.bacc as _bacc_mod
    _orig_tables = _bacc_mod.get_activation_tables

    def _pref_tables(arch):
        # act_func_set_id is positional, so keep order/length; just hide
        # Exp/Ln from every other set so both resolve to the combined one.
        t = _orig_tables(arch)
        out = {}
        for name, fns in t.items():
            if name != "natural_log_exp_and_others":
                fns = {f for f in fns if f not in (AF.Exp, AF.Ln)}
            out[name] = set(fns)
        return out

    _bacc_mod.get_activation_tables = _pref_tables
    try:
        nc.compile()
    finally:
        _bacc_mod.get_activation_tables = _orig_tables
    return nc


def _prep_shared(qkv_w, proj_w, fc1_w, fc2_w, fc1_b):
    """Host-side weight packing (shared across all cores)."""
    qkvT = np.ascontiguousarray(qkv_w.T)          # [C, 3C]: q | k | v
    wq = qkvT[:, 0:C]
    wk = qkvT[:, C:2 * C]
    wv = qkvT[:, 2 * C:3 * C]

    def pack_T(w):
        # [C, F] -> [FT, 128, CT*128]; [fj, p, ci*128+f] = w[ci*128+p, fj*128+f]
        t = w.reshape(CT, 128, FT, 128)
        return np.ascontiguousarray(
            t.transpose(2, 1, 0, 3).reshape(FT, 128, CT * 128))

    wq_packT = pack_T(wq)
    wk_packT = pack_T(wk)
    # wv: [128, ci*C + f] = wv[ci*128+p, f]
    wv_packT = np.ascontiguousarray(
        wv.reshape(CT, 128, C).transpose(1, 0, 2).reshape(128, CT * C))
    projwT = np.ascontiguousarray(proj_w.T)       # [F, C]
    fc1T = np.ascontiguousarray(fc1_w.T)          # [C, HID]
    fc1_pack = np.empty((CT, 6, 128, 512), np.float32)
    for cj in range(CT):
        for hblk in range(6):
            fc1_pack[cj, hblk] = fc1T[cj * 128:(cj + 1) * 128,
                                      hblk * 512:(hblk + 1) * 512]
    fc2T = np.ascontiguousarray(fc2_w.T)          # [HID, C]
    fc1b_cols = np.ascontiguousarray(fc1_b.reshape(HB, 128).T)
    eye = np.eye(128, dtype=np.float32)
    return dict(wq_packT=wq_packT, wk_packT=wk_packT, wv_packT=wv_packT,
                projwT=projwT, fc1w_pack=fc1_pack, fc2wT=fc2T,
                fc1b=fc1b_cols, eye=eye)


def kernel(x, policy, ln1_g, ln1_b, qkv_w, proj_w, proj_b, ln2_g, ln2_b,
           fc1_w, fc1_b, fc2_w, fc2_b):
    global LAST_RESULTS
    x = np.asarray(x, np.float32)
    policy = np.asarray(policy, np.float32)

    ln1_triv = bool(np.all(ln1_g == 1.0) and np.all(ln1_b == 0.0))
    ln2_triv = bool(np.all(ln2_g == 1.0) and np.all(ln2_b == 0.0))
    projb_triv = bool(np.all(proj_b == 0.0))
    fc2b_triv = bool(np.all(fc2_b == 0.0))
    # key compaction: each core keeps its own 512 queries as keys 0:512 plus
    # all unmasked other keys; masked non-own keys never attend anywhere
    # (their post-mask P is ~e-50) so they are dropped from K/V entirely.
    pol2 = policy[:, :, 0] > 0.5
    cols_per_core = []
    for c in range(NCORES):
        b_, qoff = c // 4, (c % 4) * QB
        own = np.arange(qoff, qoff + QB)
        other = np.concatenate([np.arange(0, qoff), np.arange(qoff + QB, N)])
        other = other[pol2[b_, other]]
        cols_per_core.append(np.concatenate([own, other]))
    kmax = max(len(cl) for cl in cols_per_core)
    kpad = ((kmax + 511) // 512) * 512

    key = (ln1_triv, ln2_triv, projb_triv, fc2b_triv, kpad)
    if key not in _prog_cache:
        _prog_cache[key] = _build_program(*key)
    nc = _prog_cache[key]
    kt_n = kpad // 128

    shared = _prep_shared(np.asarray(qkv_w, np.float32),
                          np.asarray(proj_w, np.float32),
                          np.asarray(fc1_w, np.float32),
                          np.asarray(fc2_w, np.float32),
                          np.asarray(fc1_b, np.float32))
    if not ln1_triv:
        g = np.asarray(ln1_g, np.float32).reshape(CT, 128).T
        b = np.asarray(ln1_b, np.float32).reshape(CT, 128).T
        shared["ln1gb"] = np.ascontiguousarray(np.concatenate([g, b], axis=1))
    if not ln2_triv:
        shared["ln2gb"] = np.ascontiguousarray(
            np.stack([np.asarray(ln2_g, np.float32),
                      np.asarray(ln2_b, np.float32)]))
    if not projb_triv:
        shared["projb"] = np.asarray(proj_b, np.float32).reshape(1, C)
    if not fc2b_triv:
        shared["fc2b"] = np.asarray(fc2_b, np.float32).reshape(1, C)

    in_maps = []
    for c in range(NCORES):
        b_, qoff = c // 4, (c % 4) * QB
        cols = cols_per_core[c]
        xT_c = np.zeros((C, kpad), np.float32)
        xT_c[:, :len(cols)] = x[b_].T[:, cols]
        polp = np.zeros(kpad, np.float32)
        polp[:len(cols)] = policy[b_, cols, 0]
        pol_cols = np.ascontiguousarray(polp.reshape(kt_n, 128).T)
        lnp_cols = np.ascontiguousarray(
            np.where(polp > 0.5, 0.0, MASK_NEG).astype(np.float32)
            .reshape(kt_n, 128).T)
        m = dict(shared)
        m["xT"] = xT_c
        m["x_own"] = np.ascontiguousarray(x[b_, qoff:qoff + QB])
        m["pol"] = pol_cols
        m["lnp"] = lnp_cols
        in_maps.append(m)

    res = run_bass_kernel_spmd(nc, in_maps, core_ids=list(range(NCORES)),
                               trace=TRACE, **TRACE_KWARGS)
    LAST_RESULTS = res
    out = np.empty((B, N, C), np.float32)
    for c in range(NCORES):
        b_, qoff = c // 4, (c % 4) * QB
        out[b_, qoff:qoff + QB] = res.results[c]["yout"]
    return out
